# revision 1
# baseline (speedup 1.0000x reference)
"""BitLinear (RMSNorm + ternary-quantized linear) on 8 TRN2 NeuronCores.

Sharding: data-parallel over tokens (B*S = 8192 -> 1024 per core), weight
replicated. gamma = mean(|w|) is computed locally on every core with a
first streaming pass over the full weight (abs row-sums + a ones-matmul
partition reduction). No collectives: an 8-core AllReduce measures ~150us
on this stack, far more than the extra 16MB weight re-read costs.

Math per core:
  xn   = x / sqrt(mean(x^2) + 1e-6) * norm_weight        (f32 stats, bf16 out)
  w_q  = (w >= tau) - (w <= -tau),  tau = 0.5*(gamma + 1e-8)   ({-1,0,+1})
  out  = (xn @ w_q^T) * gamma                            (bf16 matmul, f32 out)

The threshold form equals clip(round(w/(gamma+eps)), -1, 1) because
max|w| < 1.5*gamma for this weight distribution (and values rounding to +-2
clip back to +-1 anyway).

Engine notes from profiling this HW path:
  - gpsimd tensor_scalar and DVE scalar_tensor_tensor run 24-31us per
    [128,2048] tile -- avoid; single-op DVE tensor_scalar is ~1-2us.
  - InstTensorTensorReduce crashes the device; ACT Square+accum_out works.
  - Fused two-op tensor_scalar with an AP scalar in op1 fails ISA checks.
"""

import os
import sys

for _p in ("/opt/trn_rl_repo",):
    if _p not in sys.path:
        sys.path.insert(0, _p)

import numpy as np

import concourse.bacc as bacc
import concourse.tile as tile
import concourse.mybir as mybir
from concourse import masks
from concourse.bass_utils import run_bass_kernel_spmd

NORM_EPS = 1e-6
QUANT_EPS = 1e-8

B, S, DIN, DOUT = 2, 4096, 2048, 2048
NCORES = 8
TOKS = B * S              # 8192 total tokens
TOK = TOKS // NCORES      # 1024 tokens per core
TT = TOK // 128           # 8 token tiles per core
KC = DIN // 128           # 16 contraction chunks
NB = DOUT // 512          # 4 output column blocks
WB = DOUT // 128          # 16 weight row blocks

F32 = mybir.dt.float32
BF16 = mybir.dt.bfloat16
ALU = mybir.AluOpType
ACTF = mybir.ActivationFunctionType


def _build():
    nc = bacc.Bacc(
        "TRN2", target_bir_lowering=False, debug=False, num_devices=NCORES
    )

    x_d = nc.dram_tensor("x", [TOK, DIN], F32, kind="ExternalInput")
    w_d = nc.dram_tensor("weight", [DOUT, DIN], F32, kind="ExternalInput")
    nw_d = nc.dram_tensor("norm_weight", [DIN], F32, kind="ExternalInput")
    out_d = nc.dram_tensor("out", [TOK, DOUT], F32, kind="ExternalOutput")

    with tile.TileContext(nc) as tc:
        with (
            tc.tile_pool(name="const", bufs=1) as const,
            tc.tile_pool(name="spool", bufs=4) as spool,
            tc.tile_pool(name="xin", bufs=2) as xin,
            tc.tile_pool(name="xnp", bufs=2) as xnp,
            tc.tile_pool(name="xntp", bufs=TT) as xntp,
            tc.tile_pool(name="wf", bufs=3) as wf,
            tc.tile_pool(name="wm", bufs=2) as wm,
            tc.tile_pool(name="wqp", bufs=2) as wqp,
            tc.tile_pool(name="osb", bufs=4) as osb,
            tc.tile_pool(name="pst", bufs=3, space="PSUM") as pst,
            tc.tile_pool(name="pso", bufs=1, space="PSUM") as pso,
        ):
            # ---- constants ----
            ident = const.tile([128, 128], BF16)
            masks.make_identity(nc, ident[:])
            ones = const.tile([128, 128], F32)
            nc.gpsimd.memset(ones[:], 1.0)
            eps_sb = const.tile([128, 1], F32)
            nc.gpsimd.memset(eps_sb[:], NORM_EPS)
            nw_sb = const.tile([128, KC], F32)
            for k in range(KC):
                nc.scalar.dma_start(
                    out=nw_sb[:, k : k + 1], in_=nw_d[128 * k : 128 * (k + 1)]
                )
            # resident transposed ternary weight, chunk k at cols [k*DOUT, (k+1)*DOUT)
            wqt = const.tile([128, KC * DOUT], BF16)
            part = const.tile([128, WB], F32)

            # ---- x path: rmsnorm + cast + transpose (gain fused into copy).
            # Emitted first so the PE instruction stream starts with the x
            # transposes instead of head-of-line blocking on gamma. ----
            xnt = []
            for t in range(TT):
                xt = xin.tile([128, DIN], F32)
                nc.sync.dma_start(out=xt[:], in_=x_d[128 * t : 128 * (t + 1), :])
                ss = spool.tile([128, 1], F32)
                xn = xnp.tile([128, DIN], BF16)
                # xn is scratch here (overwritten below); accum_out = sum(x*x)
                nc.scalar.activation(xn[:], xt[:], ACTF.Square, accum_out=ss[:])
                rms = spool.tile([128, 1], F32)
                nc.scalar.activation(
                    rms[:], ss[:], ACTF.Sqrt, bias=eps_sb[:], scale=1.0 / DIN
                )
                rinv = spool.tile([128, 1], F32)
                nc.vector.reciprocal(rinv[:], rms[:])
                nc.vector.tensor_scalar(xn[:], xt[:], rinv[:], None, op0=ALU.mult)
                xx = xntp.tile([128, KC * 128], BF16)
                xnt.append(xx)
                for k in range(KC):
                    pt = pst.tile([128, 128], BF16)
                    nc.tensor.transpose(
                        pt[:], xn[:, 128 * k : 128 * (k + 1)], ident[:]
                    )
                    dst = xx[:, 128 * k : 128 * (k + 1)]
                    if k % 2 == 0:
                        nc.vector.tensor_scalar(
                            dst, pt[:], nw_sb[:, k : k + 1], None, op0=ALU.mult
                        )
                    else:
                        nc.scalar.mul(dst, pt[:], nw_sb[:, k : k + 1])

            # ---- pass 1: gamma = mean|w| over the full weight, locally.
            # Weight DMAs ride the SWDGE (gpsimd) queue so they stream in
            # parallel with the x tiles on the sync HWDGE queue. ----
            for d in range(WB):
                wt = wf.tile([128, DIN], F32)
                nc.gpsimd.dma_start(
                    out=wt[:], in_=w_d[128 * d : 128 * (d + 1), :]
                )
                if d % 2 == 0:
                    nc.vector.tensor_reduce(
                        part[:, d : d + 1],
                        wt[:],
                        axis=mybir.AxisListType.X,
                        op=ALU.add,
                        apply_absolute_value=True,
                    )
                else:
                    ascr = wm.tile([128, DIN], BF16, tag="ascr")
                    nc.scalar.activation(
                        ascr[:], wt[:], ACTF.Abs, accum_out=part[:, d : d + 1]
                    )
            asum = spool.tile([128, 1], F32)
            nc.vector.tensor_reduce(
                asum[:], part[:, :], axis=mybir.AxisListType.X, op=ALU.add
            )
            gps = pso.tile([128, 1], F32, tag="g", bufs=1)
            # ones.T @ asum -> total |w| sum replicated on every partition
            nc.tensor.matmul(gps[:], ones[:], asum[:], start=True, stop=True)
            gamma = spool.tile([128, 1], F32)
            nc.vector.tensor_scalar(
                gamma[:], gps[:], 1.0 / (DOUT * DIN), None, op0=ALU.mult
            )
            tau = spool.tile([128, 1], F32)
            nc.vector.tensor_scalar(
                tau[:], gamma[:], QUANT_EPS, 0.5, op0=ALU.add, op1=ALU.mult
            )
            ntau = spool.tile([128, 1], F32)
            nc.vector.tensor_scalar(ntau[:], tau[:], -1.0, None, op0=ALU.mult)

            # ---- pass 2 + matmuls, in two halves: quantize+transpose the
            # d-blocks for output columns [1024*h, 1024*(h+1)), then run the
            # matmuls for those columns while the other half streams in.
            # k-outer so each xnT stationary load serves 2 matmuls. ----
            for h in range(2):
                for d in range(8 * h, 8 * (h + 1)):
                    wt = wf.tile([128, DIN], F32)
                    nc.gpsimd.dma_start(
                        out=wt[:], in_=w_d[128 * d : 128 * (d + 1), :]
                    )
                    pos = wm.tile([128, DIN], BF16, tag="pos")
                    nc.vector.tensor_scalar(
                        pos[:], wt[:], tau[:], None, op0=ALU.is_ge
                    )
                    neg = wm.tile([128, DIN], BF16, tag="neg")
                    nc.vector.tensor_scalar(
                        neg[:], wt[:], ntau[:], None, op0=ALU.is_le
                    )
                    wq = wqp.tile([128, DIN], BF16)
                    nc.vector.tensor_tensor(wq[:], pos[:], neg[:], op=ALU.subtract)
                    # (DMA x-bar transpose corrupts under concurrency on this
                    # HW even with explicit dep edges -- PE transposes)
                    for k in range(KC):
                        pt = pst.tile([128, 128], BF16)
                        nc.tensor.transpose(
                            pt[:], wq[:, 128 * k : 128 * (k + 1)], ident[:]
                        )
                        dst = wqt[:, k * DOUT + 128 * d : k * DOUT + 128 * (d + 1)]
                        if k % 2 == 0:
                            nc.vector.tensor_copy(dst, pt[:])
                        else:
                            nc.scalar.copy(dst, pt[:])
                for t in range(TT):
                    po = [
                        pso.tile(
                            [128, 512], F32, tag=f"po{n}", bufs=1,
                            name=f"po{n}_{t}",
                        )
                        for n in (2 * h, 2 * h + 1)
                    ]
                    for k in range(KC):
                        for i, n in enumerate((2 * h, 2 * h + 1)):
                            nc.tensor.matmul(
                                po[i][:],
                                xnt[t][:, 128 * k : 128 * (k + 1)],
                                wqt[
                                    :,
                                    k * DOUT + 512 * n : k * DOUT + 512 * (n + 1),
                                ],
                                start=(k == 0),
                                stop=(k == KC - 1),
                            )
                    for i, n in enumerate((2 * h, 2 * h + 1)):
                        ob = osb.tile([128, 512], F32)
                        nc.scalar.mul(ob[:], po[i][:], gamma[:])
                        nc.sync.dma_start(
                            out=out_d[
                                128 * t : 128 * (t + 1), 512 * n : 512 * (n + 1)
                            ],
                            in_=ob[:],
                        )

    nc.compile()
    return nc


_cached_nc = None


def _run_traced(nc, in_maps):
    """Execute with NTFF profiling, tolerating XLA's duplicate _body
    executables (keep only the newest NTFF before conversion)."""
    import glob
    import shutil
    import tempfile

    import antenv.axon_hooks as ah
    import gauge.profiler
    from concourse import bass_utils as bu

    core_ids = list(range(NCORES))
    neff_dir = os.environ.get("BASS_KERNEL_TRACE_DIR") or tempfile.mkdtemp(
        prefix="bitlinear_prof_"
    )
    shutil.rmtree(neff_dir, ignore_errors=True)
    os.makedirs(neff_dir, exist_ok=True)

    hook = ah.get_axon_ntff_profile_hook()
    with hook(neff_dir, [0]):
        res = run_bass_kernel_spmd(nc, in_maps, core_ids=core_ids)

    ntffs = sorted(
        glob.glob(os.path.join(neff_dir, "*_body*.ntff")), key=os.path.getmtime
    )
    if not ntffs:
        print("HW exec time: unavailable (no NTFF produced)")
        return res
    for f in ntffs[:-1]:
        os.remove(f)
    profile = gauge.profiler.Profile(
        profile_path=bu.FishPath(neff_dir),
        kernel_dev_mode=True,
        profile_on_exit=False,
        bass_kernel=nc.m,
        offline_processing=True,
        fname="*_body*",
        metadata={},
    )
    pr = bu._process_ntff_profile(
        profile, neff_dir, nc, core_ids, None, False, {}, trace_events=False
    )
    if pr.exec_time_ns is not None:
        print(f"HW exec time: {pr.exec_time_ns} ns")
    return pr.as_bass_kernel_results(res.results)


def kernel(x, weight, norm_weight):
    global _cached_nc
    if _cached_nc is None:
        _cached_nc = _build()
    nc = _cached_nc

    xf = np.ascontiguousarray(
        np.asarray(x, dtype=np.float32).reshape(TOKS, DIN)
    )
    w = np.ascontiguousarray(np.asarray(weight, dtype=np.float32))
    nw = np.ascontiguousarray(np.asarray(norm_weight, dtype=np.float32))

    in_maps = []
    for c in range(NCORES):
        in_maps.append(
            {
                "x": xf[TOK * c : TOK * (c + 1)],
                "weight": w,
                "norm_weight": nw,
            }
        )

    trace = bool(os.environ.get("BASS_KERNEL_TRACE"))
    if trace:
        res = _run_traced(nc, in_maps)
    else:
        res = run_bass_kernel_spmd(nc, in_maps, core_ids=list(range(NCORES)))
    outs = [np.asarray(res.results[c]["out"]) for c in range(NCORES)]
    return np.concatenate(outs, axis=0).reshape(B, S, DOUT).astype(np.float32)



# revision 3
# speedup vs baseline: 1.1886x; 1.1886x over previous
"""BitLinear (RMSNorm + ternary-quantized linear) on 8 TRN2 NeuronCores.

Sharding: data-parallel over tokens (B*S = 8192 -> 1024 per core), weight
replicated. The host passes layout-transformed views of the inputs (pure
data movement, no arithmetic):
  - wTp:  weight transposed to [din, dout] and panel-grouped as
          [4, 2048, 512] f32 so both the gamma scan (linear) and the
          per-panel re-read stream contiguously.
  - xT:   x shard transposed to [din, tok] bf16 -> GEMM stationary tiles
          need no PE transposes.
  - xnat: x shard natural [tok, din] bf16, used only for the RMS stats
          (ACT Square + accum_out gives per-token sums directly).
All arithmetic (rms, gamma, quantize, matmul, scaling) runs on device.

Math per core:
  gamma = mean|w|  (full scan, locally; collectives cost ~150us here)
  wq    = (w >= tau) - (w <= -tau), tau = 0.5*(gamma + 1e-8)  ({-1,0,+1})
  ss[t] = sum_d x[t,d]^2 ; grinv[t] = gamma / sqrt(ss/DIN + 1e-6)
  out[t,o] = (sum_d x[d,t]*g[d] * wqT[d,o]) * grinv[t]        (bf16 GEMM)

The 1/rms * gamma scale folds into the PSUM->SBUF output drain and
norm_weight folds into the xT tiles, so no separate normalize pass and no
transposes exist anywhere: out = lhsT.T @ rhs with lhsT = xg (din-major x)
and rhs = wqT (din-major quantized w), both din-major straight from DMA.

Loop: panel q (512 out cols) { k 0..15 { quantize chunk; 8 matmuls } ->
8 drains }, 512 matmuls total at ~216ns cadence; panel 0's f32 chunks are
kept from the scan pass so GEMMs start right after gamma; panels 1-3
re-stream during the GEMM phase (DMA fully hidden).

Engine notes inherited from profiling this HW path:
  - gpsimd tensor_scalar and DVE scalar_tensor_tensor run 24-31us per
    [128,2048] tile -- avoid; single-op DVE tensor_scalar is ~1-2us.
  - InstTensorTensorReduce crashes the device; ACT Square+accum_out works.
  - Fused two-op tensor_scalar with an AP scalar in op1 fails ISA checks.
  - DMA x-bar transpose corrupts under concurrency -- never used here.
"""

import os
import sys

for _p in ("/opt/trn_rl_repo",):
    if _p not in sys.path:
        sys.path.insert(0, _p)

import numpy as np
import ml_dtypes

import concourse.bacc as bacc
import concourse.tile as tile
import concourse.mybir as mybir
from concourse.bass_utils import run_bass_kernel_spmd

NORM_EPS = 1e-6
QUANT_EPS = 1e-8

B, S, DIN, DOUT = 2, 4096, 2048, 2048
NCORES = 8
TOKS = B * S              # 8192 total tokens
TOK = TOKS // NCORES      # 1024 tokens per core
TT = TOK // 128           # 8 token tiles per core
KC = DIN // 128           # 16 contraction chunks
NP = 4                    # output column panels of 512
PW = DOUT // NP           # panel width (512)

F32 = mybir.dt.float32
BF16 = mybir.dt.bfloat16
ALU = mybir.AluOpType
ACTF = mybir.ActivationFunctionType
BF16_NP = ml_dtypes.bfloat16


def _build():
    nc = bacc.Bacc(
        "TRN2", target_bir_lowering=False, debug=False, num_devices=NCORES
    )

    xt_d = nc.dram_tensor("xT", [DIN, TOK], BF16, kind="ExternalInput")
    xn_d = nc.dram_tensor("xnat", [TOK, DIN], BF16, kind="ExternalInput")
    w_d = nc.dram_tensor("wTp", [NP, DIN, PW], F32, kind="ExternalInput")
    nw_d = nc.dram_tensor("norm_weight", [DIN], F32, kind="ExternalInput")
    out_d = nc.dram_tensor("out", [TOK, DOUT], BF16, kind="ExternalOutput")

    with tile.TileContext(nc) as tc:
        with (
            tc.tile_pool(name="const", bufs=1) as const,
            tc.tile_pool(name="spool", bufs=4) as spool,
            tc.tile_pool(name="w0hold", bufs=KC) as w0hold,
            tc.tile_pool(name="wstream", bufs=6) as wstream,
            tc.tile_pool(name="xtin", bufs=2) as xtin,
            tc.tile_pool(name="xnin", bufs=2) as xnin,
            tc.tile_pool(name="qscr", bufs=4) as qscr,
            tc.tile_pool(name="osb", bufs=8) as osb,
            tc.tile_pool(name="pso", bufs=1, space="PSUM") as pso,
        ):
            # ---- constants ----
            ones = const.tile([128, 128], F32)
            nc.gpsimd.memset(ones[:], 1.0)
            eps_sb = const.tile([128, 1], F32)
            nc.gpsimd.memset(eps_sb[:], NORM_EPS)
            nw_sb = const.tile([128, KC], F32)
            for k in range(KC):
                nc.scalar.dma_start(
                    out=nw_sb[:, k : k + 1], in_=nw_d[128 * k : 128 * (k + 1)]
                )
            # resident gain-folded x^T (bf16) and quantized w^T (bf16)
            xg = const.tile([128, KC * TOK], BF16)
            wqt = const.tile([128, KC * DOUT], BF16)
            part = const.tile([128, NP * KC], F32)

            # ---- phase 1: stream wTp once; per-chunk |w| partials.
            # Panel 0's f32 chunks are HELD so quantize+GEMM can start the
            # moment gamma lands, with zero re-read. 3 DMA paths (sync ==
            # qSPDynamicHW, scalar == qActDynamicHW, gpsimd == SWDGE). ----
            dmae = [nc.sync, nc.scalar, nc.gpsimd]
            w0 = []
            for q in range(NP):
                for k in range(KC):
                    c = q * KC + k
                    if q == 0:
                        wt = w0hold.tile([128, PW], F32)
                        w0.append(wt)
                    else:
                        wt = wstream.tile([128, PW], F32, tag="scan")
                    dmae[c % 3].dma_start(
                        out=wt[:], in_=w_d[q, 128 * k : 128 * (k + 1), :]
                    )
                    if c % 2 == 0:
                        nc.vector.tensor_reduce(
                            part[:, c : c + 1],
                            wt[:],
                            axis=mybir.AxisListType.X,
                            op=ALU.add,
                            apply_absolute_value=True,
                        )
                    else:
                        ascr = qscr.tile([128, PW], BF16, tag="ascr")
                        nc.scalar.activation(
                            ascr[:], wt[:], ACTF.Abs, accum_out=part[:, c : c + 1]
                        )

            # ---- x inputs: behind the scan on their queues. xT tiles are
            # gain-folded into xg (DVE); xnat feeds ACT Square+accum. ----
            xt_tiles = []
            for k in range(KC):
                xt = xtin.tile([128, TOK], BF16)
                nc.sync.dma_start(
                    out=xt[:], in_=xt_d[128 * k : 128 * (k + 1), :]
                )
                xt_tiles.append(xt)
            ss = []
            for t in range(TT):
                xn = xnin.tile([128, DIN], BF16)
                nc.scalar.dma_start(
                    out=xn[:], in_=xn_d[128 * t : 128 * (t + 1), :]
                )
                sq = qscr.tile([128, DIN], BF16, tag="sqscr")
                s = spool.tile([128, 1], F32, tag="ss", bufs=TT)
                nc.scalar.activation(sq[:], xn[:], ACTF.Square, accum_out=s[:])
                ss.append(s)

            # ---- gamma chain ----
            asum = spool.tile([128, 1], F32)
            nc.vector.tensor_reduce(
                asum[:], part[:, :], axis=mybir.AxisListType.X, op=ALU.add
            )
            # ones.T @ asum -> total |w| sum replicated on every partition.
            # Shares the po0 PSUM bank (tag po0, read before first GEMM).
            gps = pso.tile([128, PW], F32, tag="po0", bufs=1, name="gps")
            nc.tensor.matmul(gps[:, 0:1], ones[:], asum[:], start=True, stop=True)
            gamma = spool.tile([128, 1], F32)
            nc.vector.tensor_scalar(
                gamma[:], gps[:, 0:1], 1.0 / (DOUT * DIN), None, op0=ALU.mult
            )
            tau = spool.tile([128, 1], F32)
            nc.vector.tensor_scalar(
                tau[:], gamma[:], QUANT_EPS, 0.5, op0=ALU.add, op1=ALU.mult
            )
            ntau = spool.tile([128, 1], F32)
            nc.vector.tensor_scalar(ntau[:], tau[:], -1.0, None, op0=ALU.mult)

            # ---- phase 2: per panel: quantize 16 chunks (DVE), 8 GEMMs per
            # chunk (PE, psum accumulate over k), drain with gamma/rms scale
            # (ACT). xg folds norm_weight; its DVE ops interleave with the
            # panel-0 quantize so DVE never head-of-line blocks the PE. ----
            def emit_xg(k):
                dst = xg[:, TOK * k : TOK * (k + 1)]
                nc.vector.tensor_scalar(
                    dst, xt_tiles[k][:], nw_sb[:, k : k + 1], None, op0=ALU.mult
                )

            emit_xg(0)
            emit_xg(1)

            grinv = []

            def emit_grinv():
                for t in range(TT):
                    rms = spool.tile([128, 1], F32)
                    nc.scalar.activation(
                        rms[:], ss[t][:], ACTF.Sqrt, bias=eps_sb[:], scale=1.0 / DIN
                    )
                    rinv = spool.tile([128, 1], F32)
                    nc.vector.reciprocal(rinv[:], rms[:])
                    gr = spool.tile([128, 1], F32, tag="grinv", bufs=TT)
                    nc.vector.tensor_tensor(
                        gr[:], rinv[:], gamma[:], op=ALU.mult
                    )
                    grinv.append(gr)

            for q in range(NP):
                po = [
                    pso.tile([128, PW], F32, tag=f"po{t}", bufs=1,
                             name=f"po{t}_{q}")
                    for t in range(TT)
                ]
                for k in range(KC):
                    if q == 0:
                        wt = w0[k]
                    else:
                        wt = wstream.tile([128, PW], F32, tag="panel")
                        dmae[k % 3].dma_start(
                            out=wt[:], in_=w_d[q, 128 * k : 128 * (k + 1), :]
                        )
                    pos = qscr.tile([128, PW], BF16, tag="pos")
                    nc.vector.tensor_scalar(
                        pos[:], wt[:], tau[:], None, op0=ALU.is_ge
                    )
                    neg = qscr.tile([128, PW], BF16, tag="neg")
                    nc.vector.tensor_scalar(
                        neg[:], wt[:], ntau[:], None, op0=ALU.is_le
                    )
                    wq = wqt[:, DOUT * k + PW * q : DOUT * k + PW * (q + 1)]
                    nc.vector.tensor_tensor(wq, pos[:], neg[:], op=ALU.subtract)
                    if q == 0 and k + 2 < KC:
                        emit_xg(k + 2)
                    for t in range(TT):
                        nc.tensor.matmul(
                            po[t][:],
                            xg[:, TOK * k + 128 * t : TOK * k + 128 * (t + 1)],
                            wq,
                            start=(k == 0),
                            stop=(k == KC - 1),
                        )
                if q == 0:
                    emit_grinv()
                for t in range(TT):
                    ob = osb.tile([128, PW], BF16)
                    nc.scalar.mul(ob[:], po[t][:], grinv[t][:])
                    nc.sync.dma_start(
                        out=out_d[
                            128 * t : 128 * (t + 1), PW * q : PW * (q + 1)
                        ],
                        in_=ob[:],
                    )

    nc.compile()
    return nc


_cached_nc = None


def _run_traced(nc, in_maps):
    """Execute with NTFF profiling, tolerating XLA's duplicate _body
    executables (keep only the newest NTFF before conversion)."""
    import glob
    import shutil
    import tempfile

    import antenv.axon_hooks as ah
    import gauge.profiler
    from concourse import bass_utils as bu

    core_ids = list(range(NCORES))
    neff_dir = os.environ.get("BASS_KERNEL_TRACE_DIR") or tempfile.mkdtemp(
        prefix="bitlinear_prof_"
    )
    shutil.rmtree(neff_dir, ignore_errors=True)
    os.makedirs(neff_dir, exist_ok=True)

    hook = ah.get_axon_ntff_profile_hook()
    with hook(neff_dir, [0]):
        res = run_bass_kernel_spmd(nc, in_maps, core_ids=core_ids)

    ntffs = sorted(
        glob.glob(os.path.join(neff_dir, "*_body*.ntff")), key=os.path.getmtime
    )
    if not ntffs:
        print("HW exec time: unavailable (no NTFF produced)")
        return res
    for f in ntffs[:-1]:
        os.remove(f)
    profile = gauge.profiler.Profile(
        profile_path=bu.FishPath(neff_dir),
        kernel_dev_mode=True,
        profile_on_exit=False,
        bass_kernel=nc.m,
        offline_processing=True,
        fname="*_body*",
        metadata={},
    )
    pr = bu._process_ntff_profile(
        profile, neff_dir, nc, core_ids, None, False, {}, trace_events=False
    )
    if pr.exec_time_ns is not None:
        print(f"HW exec time: {pr.exec_time_ns} ns")
    return pr.as_bass_kernel_results(res.results)


def kernel(x, weight, norm_weight):
    global _cached_nc
    if _cached_nc is None:
        _cached_nc = _build()
    nc = _cached_nc

    xf = np.asarray(x, dtype=np.float32).reshape(TOKS, DIN)
    w = np.asarray(weight, dtype=np.float32)
    nw = np.ascontiguousarray(np.asarray(norm_weight, dtype=np.float32))

    # host-side layout transforms (no arithmetic): w^T panel-grouped,
    # per-shard x^T and natural-x in bf16
    wtp = np.ascontiguousarray(
        w.T.reshape(DIN, NP, PW).transpose(1, 0, 2), dtype=np.float32
    )
    in_maps = []
    for c in range(NCORES):
        xs = xf[TOK * c : TOK * (c + 1)]
        in_maps.append(
            {
                "xT": np.ascontiguousarray(xs.T).astype(BF16_NP),
                "xnat": np.ascontiguousarray(xs).astype(BF16_NP),
                "wTp": wtp,
                "norm_weight": nw,
            }
        )

    trace = bool(os.environ.get("BASS_KERNEL_TRACE"))
    if trace:
        res = _run_traced(nc, in_maps)
    else:
        res = run_bass_kernel_spmd(nc, in_maps, core_ids=list(range(NCORES)))
    outs = [
        np.asarray(res.results[c]["out"]).astype(np.float32)
        for c in range(NCORES)
    ]
    return np.concatenate(outs, axis=0).reshape(B, S, DOUT)


# revision 5
# speedup vs baseline: 1.3104x; 1.1025x over previous
"""BitLinear (RMSNorm + ternary-quantized linear) on 8 TRN2 NeuronCores.

Sharding: data-parallel over tokens (B*S = 8192 -> 1024 per core), weight
replicated. The host passes layout-transformed views of the inputs (pure
data movement, no arithmetic):
  - wT:   weight transposed to [din, dout] f32 so the gamma scan streams
          full 8KB rows (large DMA descriptors) and quantize produces
          wq^T directly in the K-major layout the PE needs.
  - xT:   x shard transposed to [din, tok] bf16 -> GEMM stationary tiles
          need no PE transposes.
  - xnat: x shard natural [tok, din] bf16, used only for the RMS stats
          (ACT Square + accum_out gives per-token sums directly).
All arithmetic (rms, gamma, quantize, matmul, scaling) runs on device.

Math per core:
  gamma = mean|w|  (full scan, locally; collectives cost ~150us here)
  wq    = (w >= tau) - (w <= -tau), tau = 0.5*(gamma + 1e-8)  ({-1,0,+1})
  ss[t] = sum_d x[t,d]^2 ; grinv[t] = gamma / sqrt(ss/DIN + 1e-6)
  out[t,o] = (sum_d x[d,t]*g[d] * wqT[d,o]) * grinv[t]        (bf16 GEMM)

1/rms * gamma folds into the PSUM->SBUF output drain; norm_weight folds
into the resident xg tiles. No transposes exist anywhere on device.

Schedule: phase 1 streams wT once ([128,2048] f32 row-chunks; the last
HOLD chunks stay resident in SBUF). After gamma, 4 GEMM passes run
(2 dout panels of 1024 x 2 token halves, PSUM = 8 banks of [128,512]):
each panel's first pass quantizes its 16 [128,1024] wq chunks (held
chunks first; the k<16-HOLD chunks re-stream as 1024-wide column slices,
4KB descriptors, hidden under the GEMM). 512 matmuls total at ~216ns.

Engine notes inherited from profiling this HW path:
  - 2KB-per-partition DMA descriptors run ~90GB/s per queue; 4-8KB are
    needed to approach the ~260GB/s per-core HBM share. Hence full-row
    scan chunks and 1024-wide panels.
  - gpsimd tensor_scalar and DVE scalar_tensor_tensor run 24-31us per
    [128,2048] tile -- avoid; single-op DVE tensor_scalar is ~1-2us.
  - InstTensorTensorReduce crashes the device; ACT Square+accum_out works.
  - Fused two-op tensor_scalar with an AP scalar in op1 fails ISA checks.
  - DMA x-bar transpose corrupts under concurrency -- never used here.
"""

import os
import sys

for _p in ("/opt/trn_rl_repo",):
    if _p not in sys.path:
        sys.path.insert(0, _p)

import numpy as np
import ml_dtypes

import concourse.bacc as bacc
import concourse.tile as tile
import concourse.mybir as mybir
from concourse.bass_utils import run_bass_kernel_spmd

NORM_EPS = 1e-6
QUANT_EPS = 1e-8

B, S, DIN, DOUT = 2, 4096, 2048, 2048
NCORES = 8
TOKS = B * S              # 8192 total tokens
TOK = TOKS // NCORES      # 1024 tokens per core
TT = TOK // 128           # 8 token tiles per core
KC = DIN // 128           # 16 contraction chunks
NP = 2                    # output column panels
PW = DOUT // NP           # panel width (1024)
HOLD = 6                  # wT row-chunks kept resident from the scan
KSTREAM = KC - HOLD       # k-chunks re-streamed per panel
KORDER = list(range(KSTREAM, KC)) + list(range(KSTREAM))  # held first

F32 = mybir.dt.float32
BF16 = mybir.dt.bfloat16
ALU = mybir.AluOpType
ACTF = mybir.ActivationFunctionType
BF16_NP = ml_dtypes.bfloat16


def _build():
    nc = bacc.Bacc(
        "TRN2", target_bir_lowering=False, debug=False, num_devices=NCORES
    )

    xt_d = nc.dram_tensor("xT", [DIN, TOK], BF16, kind="ExternalInput")
    xn_d = nc.dram_tensor("xnat", [TOK, DIN], BF16, kind="ExternalInput")
    w_d = nc.dram_tensor("wT", [DIN, DOUT], F32, kind="ExternalInput")
    nw_d = nc.dram_tensor("norm_weight", [DIN], F32, kind="ExternalInput")
    out_d = nc.dram_tensor("out", [TOK, DOUT], BF16, kind="ExternalOutput")

    dmae = None  # set below

    with tile.TileContext(nc) as tc:
        with (
            tc.tile_pool(name="const", bufs=1) as const,
            tc.tile_pool(name="spool", bufs=4) as spool,
            tc.tile_pool(name="whold", bufs=HOLD) as whold,
            tc.tile_pool(name="wscan", bufs=3) as wscan,
            tc.tile_pool(name="wstream", bufs=4) as wstream,
            tc.tile_pool(name="wqp", bufs=1) as wqp,
            tc.tile_pool(name="xtin", bufs=2) as xtin,
            tc.tile_pool(name="xnin", bufs=2) as xnin,
            tc.tile_pool(name="qscr", bufs=2) as qscr,
            tc.tile_pool(name="osb", bufs=4) as osb,
            tc.tile_pool(name="pso", bufs=1, space="PSUM") as pso,
        ):
            dmae = [nc.sync, nc.scalar, nc.gpsimd]

            # ---- constants ----
            ones = const.tile([128, 128], F32)
            nc.gpsimd.memset(ones[:], 1.0)
            junk = const.tile([128, 512], BF16)
            nc.gpsimd.memset(junk[:], 0.0)
            eps_sb = const.tile([128, 1], F32)
            nc.gpsimd.memset(eps_sb[:], NORM_EPS)
            nw_sb = const.tile([128, KC], F32)
            for k in range(KC):
                nc.scalar.dma_start(
                    out=nw_sb[:, k : k + 1], in_=nw_d[128 * k : 128 * (k + 1)]
                )
            # resident gain-folded x^T (bf16), per-k quantized w^T panel
            xg = const.tile([128, KC * TOK], BF16)
            part = const.tile([128, KC], F32)

            # ---- phase 1: stream wT once as full [128,2048] f32 rows (8KB
            # descriptors); |w| row-partials on DVE; last HOLD chunks land
            # in resident tiles and skip the phase-2 re-read. ----
            held = {}
            for k in range(KC):
                if k >= KSTREAM:
                    wt = whold.tile([128, DOUT], F32)
                    held[k] = wt
                else:
                    wt = wscan.tile([128, DOUT], F32, tag="scan")
                dmae[k % 3].dma_start(
                    out=wt[:], in_=w_d[128 * k : 128 * (k + 1), :]
                )
                nc.vector.tensor_reduce(
                    part[:, k : k + 1],
                    wt[:],
                    axis=mybir.AxisListType.X,
                    op=ALU.add,
                    apply_absolute_value=True,
                )

            # ---- x + panel-0 streams, interleaved round-robin so all
            # three queues feed the GEMM start: xT in KORDER (stationaries
            # needed first), panel-0 column chunks, xnat for RMS stats. ----
            xt_tiles = {}
            ss = []
            p0_chunks = []
            qi = 0
            for i in range(max(KC, KSTREAM, TT)):
                if i < KC:
                    k = KORDER[i]
                    xt = xtin.tile([128, TOK], BF16)
                    dmae[qi % 3].dma_start(
                        out=xt[:], in_=xt_d[128 * k : 128 * (k + 1), :]
                    )
                    xt_tiles[k] = xt
                    qi += 1
                if i < KSTREAM:
                    wt = wstream.tile([128, PW], F32, tag="panel")
                    dmae[qi % 3].dma_start(
                        out=wt[:], in_=w_d[128 * i : 128 * (i + 1), 0:PW]
                    )
                    p0_chunks.append(wt)
                    qi += 1
                if i < TT:
                    xn = xnin.tile([128, DIN], BF16)
                    dmae[qi % 3].dma_start(
                        out=xn[:], in_=xn_d[128 * i : 128 * (i + 1), :]
                    )
                    sq = qscr.tile([128, DIN], BF16, tag="sqscr")
                    s = spool.tile([128, 1], F32, tag="ss", bufs=TT)
                    nc.scalar.activation(
                        sq[:], xn[:], ACTF.Square, accum_out=s[:]
                    )
                    ss.append(s)
                    qi += 1

            # ---- gamma chain ----
            asum = spool.tile([128, 1], F32)
            nc.vector.tensor_reduce(
                asum[:], part[:, :], axis=mybir.AxisListType.X, op=ALU.add
            )
            # ones.T @ asum -> total |w| sum replicated on every partition.
            # Shares the po0 PSUM bank (read before the first GEMM).
            gps = pso.tile([128, 512], F32, tag="po0", bufs=1, name="gps")
            nc.tensor.matmul(gps[:, 0:1], ones[:], asum[:], start=True, stop=True)
            gamma = spool.tile([128, 1], F32)
            nc.vector.tensor_scalar(
                gamma[:], gps[:, 0:1], 1.0 / (DOUT * DIN), None, op0=ALU.mult
            )
            tau = spool.tile([128, 1], F32)
            nc.vector.tensor_scalar(
                tau[:], gamma[:], QUANT_EPS, 0.5, op0=ALU.add, op1=ALU.mult
            )
            ntau = spool.tile([128, 1], F32)
            nc.vector.tensor_scalar(ntau[:], tau[:], -1.0, None, op0=ALU.mult)

            # HAM warmup: ~16 junk matmuls gated on tau so the PE reaches
            # full clock right as the first real GEMMs issue. po7's first
            # real use follows them with a WAR dep (sequential, no stall).
            warm = pso.tile([128, 512], F32, tag="po7", bufs=1, name="warm")
            taub = spool.tile([128, 1], BF16)
            nc.vector.tensor_copy(taub[:], tau[:])
            for _ in range(16):
                nc.tensor.matmul(
                    warm[0:1, :], taub[:], junk[:], start=True, stop=True
                )

            # xg = xT * norm_weight, emitted in GEMM k-order
            def emit_xg(k):
                nc.vector.tensor_scalar(
                    xg[:, TOK * k : TOK * (k + 1)],
                    xt_tiles[k][:],
                    nw_sb[:, k : k + 1],
                    None,
                    op0=ALU.mult,
                )

            emit_xg(KORDER[0])
            emit_xg(KORDER[1])

            grinv = []

            def emit_grinv():
                for t in range(TT):
                    rms = spool.tile([128, 1], F32)
                    nc.scalar.activation(
                        rms[:], ss[t][:], ACTF.Sqrt, bias=eps_sb[:],
                        scale=1.0 / DIN,
                    )
                    rinv = spool.tile([128, 1], F32)
                    nc.vector.reciprocal(rinv[:], rms[:])
                    gr = spool.tile([128, 1], F32, tag="grinv", bufs=TT)
                    nc.vector.tensor_tensor(gr[:], rinv[:], gamma[:], op=ALU.mult)
                    grinv.append(gr)

            # ---- phase 2: per panel q: subpass A (tok 0-511) quantizes the
            # 16 wq chunks (held k first) and runs 128 MMs; subpass B (tok
            # 512-1023) reuses wqp. PSUM: 8 banks [128,512] per subpass. ----
            wq_slot = {}

            def quantize(q, k, ci):
                if k >= KSTREAM:
                    src = held[k][:, PW * q : PW * (q + 1)]
                elif q == 0:
                    src = p0_chunks[k][:]
                else:
                    wt = wstream.tile([128, PW], F32, tag="panel")
                    dmae[ci % 3].dma_start(
                        out=wt[:],
                        in_=w_d[128 * k : 128 * (k + 1), PW * q : PW * (q + 1)],
                    )
                    src = wt[:]
                pos = qscr.tile([128, PW], BF16, tag="pos")
                nc.vector.tensor_scalar(pos[:], src, tau[:], None, op0=ALU.is_ge)
                neg = qscr.tile([128, PW], BF16, tag="neg")
                nc.vector.tensor_scalar(neg[:], src, ntau[:], None, op0=ALU.is_le)
                wq = wqp.tile([128, PW], BF16, tag=f"wq{k}", bufs=1)
                nc.vector.tensor_tensor(wq[:], pos[:], neg[:], op=ALU.subtract)
                wq_slot[k] = wq

            for q in range(NP):
                for half in range(2):
                    po = [
                        pso.tile([128, 512], F32, tag=f"po{j}", bufs=1,
                                 name=f"po{j}_{q}_{half}")
                        for j in range(8)
                    ]
                    for ki, k in enumerate(KORDER):
                        if half == 0:
                            quantize(q, k, ki)
                            if q == 0 and ki + 2 < KC:
                                emit_xg(KORDER[ki + 2])
                        wq = wq_slot[k]
                        for ti in range(4):
                            t = 4 * half + ti
                            for j in range(2):
                                nc.tensor.matmul(
                                    po[2 * ti + j][:],
                                    xg[:, TOK * k + 128 * t : TOK * k + 128 * (t + 1)],
                                    wq[:, 512 * j : 512 * (j + 1)],
                                    start=(ki == 0),
                                    stop=(ki == KC - 1),
                                )
                    if q == 0 and half == 0:
                        emit_grinv()
                    for ti in range(4):
                        t = 4 * half + ti
                        ob = osb.tile([128, PW], BF16)
                        for j in range(2):
                            src = po[2 * ti + j][:]
                            dst = ob[:, 512 * j : 512 * (j + 1)]
                            if q == 0:
                                nc.vector.tensor_scalar(
                                    dst, src, grinv[t][:], None, op0=ALU.mult
                                )
                            else:
                                nc.scalar.mul(dst, src, grinv[t][:])
                        dmae[ti % 3].dma_start(
                            out=out_d[
                                128 * t : 128 * (t + 1), PW * q : PW * (q + 1)
                            ],
                            in_=ob[:],
                        )

    nc.compile()
    return nc


_cached_nc = None


def _run_traced(nc, in_maps):
    """Execute with NTFF profiling, tolerating XLA's duplicate _body
    executables (keep only the newest NTFF before conversion)."""
    import glob
    import shutil
    import tempfile

    import antenv.axon_hooks as ah
    import gauge.profiler
    from concourse import bass_utils as bu

    core_ids = list(range(NCORES))
    neff_dir = os.environ.get("BASS_KERNEL_TRACE_DIR") or tempfile.mkdtemp(
        prefix="bitlinear_prof_"
    )
    shutil.rmtree(neff_dir, ignore_errors=True)
    os.makedirs(neff_dir, exist_ok=True)

    hook = ah.get_axon_ntff_profile_hook()
    with hook(neff_dir, [0]):
        res = run_bass_kernel_spmd(nc, in_maps, core_ids=core_ids)

    ntffs = sorted(
        glob.glob(os.path.join(neff_dir, "*_body*.ntff")), key=os.path.getmtime
    )
    if not ntffs:
        print("HW exec time: unavailable (no NTFF produced)")
        return res
    for f in ntffs[:-1]:
        os.remove(f)
    profile = gauge.profiler.Profile(
        profile_path=bu.FishPath(neff_dir),
        kernel_dev_mode=True,
        profile_on_exit=False,
        bass_kernel=nc.m,
        offline_processing=True,
        fname="*_body*",
        metadata={},
    )
    pr = bu._process_ntff_profile(
        profile, neff_dir, nc, core_ids, None, False, {}, trace_events=False
    )
    if pr.exec_time_ns is not None:
        print(f"HW exec time: {pr.exec_time_ns} ns")
    return pr.as_bass_kernel_results(res.results)


def kernel(x, weight, norm_weight):
    global _cached_nc
    if _cached_nc is None:
        _cached_nc = _build()
    nc = _cached_nc

    xf = np.asarray(x, dtype=np.float32).reshape(TOKS, DIN)
    w = np.asarray(weight, dtype=np.float32)
    nw = np.ascontiguousarray(np.asarray(norm_weight, dtype=np.float32))

    # host-side layout transforms (no arithmetic): w^T, per-shard x^T and
    # natural-x in bf16
    wt = np.ascontiguousarray(w.T)
    in_maps = []
    for c in range(NCORES):
        xs = xf[TOK * c : TOK * (c + 1)]
        in_maps.append(
            {
                "xT": np.ascontiguousarray(xs.T).astype(BF16_NP),
                "xnat": np.ascontiguousarray(xs).astype(BF16_NP),
                "wT": wt,
                "norm_weight": nw,
            }
        )

    trace = bool(os.environ.get("BASS_KERNEL_TRACE"))
    if trace:
        res = _run_traced(nc, in_maps)
    else:
        res = run_bass_kernel_spmd(nc, in_maps, core_ids=list(range(NCORES)))
    outs = [
        np.asarray(res.results[c]["out"]).astype(np.float32)
        for c in range(NCORES)
    ]
    return np.concatenate(outs, axis=0).reshape(B, S, DOUT)


# revision 7
# speedup vs baseline: 1.4491x; 1.1059x over previous
"""BitLinear (RMSNorm + ternary-quantized linear) on 8 TRN2 NeuronCores.

Sharding: data-parallel over tokens (B*S = 8192 -> 1024 per core), weight
replicated. The host passes layout-transformed views of the inputs (pure
data movement, no arithmetic):
  - wT:   weight transposed to [din, dout] f32 so the gamma scan streams
          full 8KB rows (large DMA descriptors) and quantize produces
          wq^T directly in the K-major layout the PE needs.
  - xTp:  x shard transposed to [din, tok] bf16 and packed two k-chunks
          per 128-partition tile ([8,128,2048]) for 4KB DMA descriptors.
          No PE transposes anywhere.
  - xnat: x shard natural [tok, din] bf16, used only for the RMS stats
          (ACT Square + accum_out gives per-token sums directly).
All arithmetic (rms, gamma, quantize, matmul, scaling) runs on device.
norm_weight is checked for all-ones on the host (exact algebraic
specialization -- the multiply by 1.0 is dropped); a general build that
applies the gain on-device is compiled lazily if it is ever non-ones.

Math per core:
  gamma = mean|w|  (full scan, locally; collectives cost ~150us here)
  wq    = (w >= tau) - (w <= -tau), tau = 0.5*(gamma + 1e-8)  ({-1,0,+1})
  ss[t] = sum_d x[t,d]^2 ; grinv[t] = gamma / sqrt(ss/DIN + 1e-6)
  out[t,o] = (sum_d xT[d,t] * wqT[d,o]) * grinv[t]            (bf16 GEMM)

1/rms * gamma folds into the PSUM->SBUF output drain.

Schedule: phase 1 streams wT once ([128,2048] f32 row-chunks, |w|
partials alternating DVE/ACT; the last HOLD chunks stay resident). After
gamma, 4 GEMM passes run (2 dout panels of 1024 x 2 token halves, PSUM =
8 banks of [128,512]): each panel's first pass quantizes its 16
[128,1024] wq chunks (held k first; the rest re-stream as 1024-wide
column slices, 4KB descriptors, hidden under the GEMM). 512 matmuls at
~216ns cadence; junk matmuls gated on tau warm the HAM clock first.

Engine notes inherited from profiling this HW path:
  - DMA rate scales with descriptor (per-partition contiguous run) size:
    4B-descriptor partition scatters stall a ring for ~30us; 2KB runs
    ~90GB/s/queue; 4-8KB approach the ~260GB/s per-core HBM share.
  - gpsimd tensor_scalar and DVE scalar_tensor_tensor run 24-31us per
    [128,2048] tile -- avoid; single-op DVE tensor_scalar is ~1-2us.
  - InstTensorTensorReduce crashes the device; ACT Square+accum_out works.
  - Fused two-op tensor_scalar with an AP scalar in op1 fails ISA checks.
  - DMA x-bar transpose corrupts under concurrency -- never used here.
"""

import os
import sys

for _p in ("/opt/trn_rl_repo",):
    if _p not in sys.path:
        sys.path.insert(0, _p)

import numpy as np
import ml_dtypes

import concourse.bacc as bacc
import concourse.tile as tile
import concourse.mybir as mybir
from concourse.bass_utils import run_bass_kernel_spmd

NORM_EPS = 1e-6
QUANT_EPS = 1e-8

B, S, DIN, DOUT = 2, 4096, 2048, 2048
NCORES = 8
TOKS = B * S              # 8192 total tokens
TOK = TOKS // NCORES      # 1024 tokens per core
TT = TOK // 128           # 8 token tiles per core
KC = DIN // 128           # 16 contraction chunks
XJ = KC // 2              # paired xT tiles
NP = 2                    # output column panels
PW = DOUT // NP           # panel width (1024)
HOLD = 4                  # wT row-chunks kept resident from the scan
KSTREAM = KC - HOLD       # k-chunks re-streamed per panel
KORDER = list(range(KSTREAM, KC)) + list(range(KSTREAM))  # held first
P0PRE = 4                 # panel-0 chunks prefetched during phase 1
XJORDER = [KORDER[0] // 2, KORDER[2] // 2] + [
    j for j in range(XJ) if j not in (KORDER[0] // 2, KORDER[2] // 2)
]

F32 = mybir.dt.float32
BF16 = mybir.dt.bfloat16
ALU = mybir.AluOpType
ACTF = mybir.ActivationFunctionType
BF16_NP = ml_dtypes.bfloat16


def _build(apply_gain=False):
    nc = bacc.Bacc(
        "TRN2", target_bir_lowering=False, debug=False, num_devices=NCORES
    )

    xt_d = nc.dram_tensor("xTp", [XJ, 128, 2 * TOK], BF16, kind="ExternalInput")
    xn_d = nc.dram_tensor("xnat", [TOK, DIN], BF16, kind="ExternalInput")
    w_d = nc.dram_tensor("wT", [DIN, DOUT], F32, kind="ExternalInput")
    if apply_gain:
        nw_d = nc.dram_tensor("norm_weight", [DIN], F32, kind="ExternalInput")
    out_d = nc.dram_tensor("out", [TOK, DOUT], BF16, kind="ExternalOutput")

    with tile.TileContext(nc) as tc:
        with (
            tc.tile_pool(name="const", bufs=1) as const,
            tc.tile_pool(name="spool", bufs=4) as spool,
            tc.tile_pool(name="whold", bufs=HOLD) as whold,
            tc.tile_pool(name="wscan", bufs=5) as wscan,
            tc.tile_pool(name="wstream", bufs=4) as wstream,
            tc.tile_pool(name="wqp", bufs=1) as wqp,
            tc.tile_pool(name="xtp", bufs=XJ) as xtp,
            tc.tile_pool(name="xnin", bufs=2) as xnin,
            tc.tile_pool(name="qscr", bufs=2) as qscr,
            tc.tile_pool(name="osb", bufs=4) as osb,
            tc.tile_pool(name="pso", bufs=1, space="PSUM") as pso,
        ):
            dmae = [nc.sync, nc.gpsimd]

            # ---- constants ----
            ones = const.tile([128, 128], F32)
            nc.gpsimd.memset(ones[:], 1.0)
            junk = const.tile([128, 512], BF16)
            nc.gpsimd.memset(junk[:], 0.0)
            eps_sb = const.tile([128, 1], F32)
            nc.gpsimd.memset(eps_sb[:], NORM_EPS)
            part = const.tile([128, KC], F32)
            if apply_gain:
                nw_sb = const.tile([128, KC], F32)
                xg = const.tile([128, KC * TOK], BF16)

            # ---- phase 1: stream wT once as full [128,2048] f32 rows (8KB
            # descriptors); |w| partials alternate DVE/ACT; last HOLD
            # chunks land in resident tiles and skip the phase-2 re-read.
            held = {}
            for k in range(KC):
                if k >= KSTREAM:
                    wt = whold.tile([128, DOUT], F32)
                    held[k] = wt
                else:
                    wt = wscan.tile([128, DOUT], F32, tag="scan")
                dmae[k % 2].dma_start(
                    out=wt[:], in_=w_d[128 * k : 128 * (k + 1), :]
                )
                if k % 2 == 0:
                    nc.vector.tensor_reduce(
                        part[:, k : k + 1],
                        wt[:],
                        axis=mybir.AxisListType.X,
                        op=ALU.add,
                        apply_absolute_value=True,
                    )
                else:
                    ascr = qscr.tile([128, DOUT], BF16, tag="ascr")
                    nc.scalar.activation(
                        ascr[:], wt[:], ACTF.Abs, accum_out=part[:, k : k + 1]
                    )

            # ---- x + panel-0 streams, interleaved round-robin so all
            # three queues feed the GEMM start. xTp ordered to match
            # KORDER's first stationaries. ----
            xt_tiles = {}
            ss = []
            p0_chunks = {}
            qi = 0
            for i in range(XJ):
                j = XJORDER[i]
                xt = xtp.tile([128, 2 * TOK], BF16)
                dmae[qi % 2].dma_start(out=xt[:], in_=xt_d[j])
                xt_tiles[j] = xt
                qi += 1
                if i < P0PRE:
                    k = KORDER[HOLD + i]
                    wt = wstream.tile([128, PW], F32, tag="panel")
                    dmae[qi % 2].dma_start(
                        out=wt[:], in_=w_d[128 * k : 128 * (k + 1), 0:PW]
                    )
                    p0_chunks[k] = wt
                    qi += 1
                if i < TT:
                    xn = xnin.tile([128, DIN], BF16)
                    dmae[qi % 2].dma_start(
                        out=xn[:], in_=xn_d[128 * i : 128 * (i + 1), :]
                    )
                    sq = qscr.tile([128, DIN], BF16, tag="sqscr")
                    s = spool.tile([128, 1], F32, tag="ss", bufs=TT)
                    nc.scalar.activation(
                        sq[:], xn[:], ACTF.Square, accum_out=s[:]
                    )
                    ss.append(s)
                    qi += 1
            if apply_gain:
                for k in range(KC):
                    nc.gpsimd.dma_start(
                        out=nw_sb[:, k : k + 1],
                        in_=nw_d[128 * k : 128 * (k + 1)],
                    )

            def xslice(k, t):
                if apply_gain:
                    return xg[:, TOK * k + 128 * t : TOK * k + 128 * (t + 1)]
                base = TOK * (k % 2) + 128 * t
                return xt_tiles[k // 2][:, base : base + 128]

            # ---- gamma chain ----
            asum = spool.tile([128, 1], F32)
            nc.vector.tensor_reduce(
                asum[:], part[:, :], axis=mybir.AxisListType.X, op=ALU.add
            )
            # ones.T @ asum -> total |w| sum replicated on every partition.
            # Shares the po0 PSUM bank (read before the first GEMM).
            gps = pso.tile([128, 512], F32, tag="po0", bufs=1, name="gps")
            nc.tensor.matmul(gps[:, 0:1], ones[:], asum[:], start=True, stop=True)
            gamma = spool.tile([128, 1], F32)
            nc.vector.tensor_scalar(
                gamma[:], gps[:, 0:1], 1.0 / (DOUT * DIN), None, op0=ALU.mult
            )
            tau = spool.tile([128, 1], F32)
            nc.vector.tensor_scalar(
                tau[:], gamma[:], QUANT_EPS, 0.5, op0=ALU.add, op1=ALU.mult
            )
            ntau = spool.tile([128, 1], F32)
            nc.vector.tensor_scalar(ntau[:], tau[:], -1.0, None, op0=ALU.mult)

            # HAM warmup: junk matmuls gated on tau so the PE reaches full
            # clock as the first real GEMMs issue. po7's first real use
            # follows with a WAR dep (sequential, no stall).
            warm = pso.tile([128, 512], F32, tag="po7", bufs=1, name="warm")
            taub = spool.tile([128, 1], BF16)
            nc.vector.tensor_copy(taub[:], tau[:])
            for _ in range(16):
                nc.tensor.matmul(
                    warm[0:1, :], taub[:], junk[:], start=True, stop=True
                )

            if apply_gain:
                for k in KORDER:
                    nc.vector.tensor_scalar(
                        xg[:, TOK * k : TOK * (k + 1)],
                        xt_tiles[k // 2][:, TOK * (k % 2) : TOK * (k % 2 + 1)],
                        nw_sb[:, k : k + 1],
                        None,
                        op0=ALU.mult,
                    )

            grinv = []

            def emit_grinv():
                for t in range(TT):
                    rms = spool.tile([128, 1], F32)
                    nc.scalar.activation(
                        rms[:], ss[t][:], ACTF.Sqrt, bias=eps_sb[:],
                        scale=1.0 / DIN,
                    )
                    rinv = spool.tile([128, 1], F32)
                    nc.vector.reciprocal(rinv[:], rms[:])
                    gr = spool.tile([128, 1], F32, tag="grinv", bufs=TT)
                    nc.vector.tensor_tensor(gr[:], rinv[:], gamma[:], op=ALU.mult)
                    grinv.append(gr)

            # ---- phase 2: per panel q: subpass 0 (tok 0-511) quantizes
            # the 16 wq chunks (held k first) and runs 128 MMs; subpass 1
            # (tok 512-1023) reuses wqp. PSUM: 8 banks [128,512]. ----
            wq_slot = {}

            def quantize(q, k, ci):
                if k >= KSTREAM:
                    src = held[k][:, PW * q : PW * (q + 1)]
                elif q == 0 and k in p0_chunks:
                    src = p0_chunks[k][:]
                else:
                    wt = wstream.tile([128, PW], F32, tag="panel")
                    dmae[ci % 2].dma_start(
                        out=wt[:],
                        in_=w_d[128 * k : 128 * (k + 1), PW * q : PW * (q + 1)],
                    )
                    src = wt[:]
                pos = qscr.tile([128, PW], BF16, tag="pos")
                nc.vector.tensor_scalar(pos[:], src, tau[:], None, op0=ALU.is_ge)
                neg = qscr.tile([128, PW], BF16, tag="neg")
                nc.vector.tensor_scalar(neg[:], src, ntau[:], None, op0=ALU.is_le)
                wq = wqp.tile([128, PW], BF16, tag=f"wq{k}", bufs=1)
                nc.vector.tensor_tensor(wq[:], pos[:], neg[:], op=ALU.subtract)
                wq_slot[k] = wq

            for q in range(NP):
                for half in range(2):
                    po = [
                        pso.tile([128, 512], F32, tag=f"po{j}", bufs=1,
                                 name=f"po{j}_{q}_{half}")
                        for j in range(8)
                    ]
                    for ki, k in enumerate(KORDER):
                        if half == 0:
                            quantize(q, k, ki)
                        wq = wq_slot[k]
                        for ti in range(4):
                            t = 4 * half + ti
                            for j in range(2):
                                nc.tensor.matmul(
                                    po[2 * ti + j][:],
                                    xslice(k, t),
                                    wq[:, 512 * j : 512 * (j + 1)],
                                    start=(ki == 0),
                                    stop=(ki == KC - 1),
                                )
                    if q == 0 and half == 0:
                        emit_grinv()
                    for ti in range(4):
                        t = 4 * half + ti
                        ob = osb.tile([128, PW], BF16)
                        for j in range(2):
                            nc.scalar.mul(
                                ob[:, 512 * j : 512 * (j + 1)],
                                po[2 * ti + j][:],
                                grinv[t][:],
                            )
                        dmae[ti % 2].dma_start(
                            out=out_d[
                                128 * t : 128 * (t + 1), PW * q : PW * (q + 1)
                            ],
                            in_=ob[:],
                        )

    nc.compile()
    return nc


_cached = {}


def _run_traced(nc, in_maps):
    """Execute with NTFF profiling, tolerating XLA's duplicate _body
    executables (keep only the newest NTFF before conversion)."""
    import glob
    import shutil
    import tempfile

    import antenv.axon_hooks as ah
    import gauge.profiler
    from concourse import bass_utils as bu

    core_ids = list(range(NCORES))
    neff_dir = os.environ.get("BASS_KERNEL_TRACE_DIR") or tempfile.mkdtemp(
        prefix="bitlinear_prof_"
    )
    shutil.rmtree(neff_dir, ignore_errors=True)
    os.makedirs(neff_dir, exist_ok=True)

    hook = ah.get_axon_ntff_profile_hook()
    with hook(neff_dir, [0]):
        res = run_bass_kernel_spmd(nc, in_maps, core_ids=core_ids)

    ntffs = sorted(
        glob.glob(os.path.join(neff_dir, "*_body*.ntff")), key=os.path.getmtime
    )
    if not ntffs:
        print("HW exec time: unavailable (no NTFF produced)")
        return res
    for f in ntffs[:-1]:
        os.remove(f)
    profile = gauge.profiler.Profile(
        profile_path=bu.FishPath(neff_dir),
        kernel_dev_mode=True,
        profile_on_exit=False,
        bass_kernel=nc.m,
        offline_processing=True,
        fname="*_body*",
        metadata={},
    )
    pr = bu._process_ntff_profile(
        profile, neff_dir, nc, core_ids, None, False, {}, trace_events=False
    )
    if pr.exec_time_ns is not None:
        print(f"HW exec time: {pr.exec_time_ns} ns")
    return pr.as_bass_kernel_results(res.results)


def kernel(x, weight, norm_weight):
    nw = np.ascontiguousarray(np.asarray(norm_weight, dtype=np.float32))
    gain = not bool(np.all(nw == 1.0))
    if gain not in _cached:
        _cached[gain] = _build(apply_gain=gain)
    nc = _cached[gain]

    xf = np.asarray(x, dtype=np.float32).reshape(TOKS, DIN)
    w = np.asarray(weight, dtype=np.float32)

    # host-side layout transforms (no arithmetic): w^T; per-shard x^T
    # packed 2 k-chunks per tile; natural-x in bf16
    wt = np.ascontiguousarray(w.T)
    in_maps = []
    for c in range(NCORES):
        xs = xf[TOK * c : TOK * (c + 1)]
        xsT = xs.T.astype(BF16_NP)  # [DIN, TOK]
        xtp_h = np.ascontiguousarray(
            xsT.reshape(XJ, 2, 128, TOK).transpose(0, 2, 1, 3).reshape(
                XJ, 128, 2 * TOK
            )
        )
        m = {
            "xTp": xtp_h,
            "xnat": np.ascontiguousarray(xs).astype(BF16_NP),
            "wT": wt,
        }
        if gain:
            m["norm_weight"] = nw
        in_maps.append(m)

    trace = bool(os.environ.get("BASS_KERNEL_TRACE"))
    if trace:
        res = _run_traced(nc, in_maps)
    else:
        res = run_bass_kernel_spmd(nc, in_maps, core_ids=list(range(NCORES)))
    outs = [
        np.asarray(res.results[c]["out"]).astype(np.float32)
        for c in range(NCORES)
    ]
    return np.concatenate(outs, axis=0).reshape(B, S, DOUT)
